# revision 9
# baseline (speedup 1.0000x reference)
"""DSSM (conv + kmax-pooling + sem) Trainium2 kernel.

Data-parallel over batch across 8 NeuronCores; weights and the embedding
table (host-cast to fp16) are replicated per core.  Each core:
  1. two-stage embedding gather:
     a) per-vocab-range `dma_gather` HBM->SBUF (int16 indices; tokens
        range-sorted host-side, ranges padded to 128 with token 0),
     b) one SBUF-source transposed `dma_gather` whose permutation indices
        simultaneously undo the range-sort and transpose to [H, tok] with
        tokens contiguous per batch row,
  2. conv1d == 6 accumulated matmuls (2 H-tiles x 3 shifts) in fp16,
  3. k-max pooling via DVE max/max_index (top-8) + branchless rank/select
     to restore positional order (bias/tanh folded to after the pooling --
     both are monotonic along L),
  4. sem layer as fp32 matmul, tanh on ACT, gamma projection via a
     ones-vector matmul partition-reduce.
"""

from contextlib import ExitStack

import numpy as np

import concourse.bass as bass
import concourse.tile as tile
from concourse import bacc, mybir
from concourse.bass_utils import run_bass_kernel_spmd

# problem dims
B, QL, DL = 512, 64, 256
V, H = 100000, 256
CO, KS = 512, 3
KMAX, LAT = 3, 128
N_CORES = 8
B_SH = B // N_CORES          # 64 rows per core
RANGE = 1 << 15              # int16-addressable vocab range
NRANGES = -(-V // RANGE)
DOC_ROWS_PER_CHUNK = 16
CHUNK_TOK = 4096             # query: 64x64; doc chunk: 16x256

F32 = mybir.dt.float32
F16 = mybir.dt.float16
I16 = mybir.dt.int16
U32 = mybir.dt.uint32


def _r128(n):
    return -(-n // 128) * 128


def _core_chunk_tokens(query_, doc_, core, b_sh=B_SH):
    """Token id arrays (CHUNK_TOK each) for each chunk of one core."""
    qs = np.asarray(query_[core * b_sh:(core + 1) * b_sh]).astype(np.int64)
    ds = np.asarray(doc_[core * b_sh:(core + 1) * b_sh]).astype(np.int64)
    chunks = [qs.ravel()]
    for r0 in range(0, b_sh, DOC_ROWS_PER_CHUNK):
        chunks.append(ds[r0:r0 + DOC_ROWS_PER_CHUNK].ravel())
    return chunks


def plan_quotas(query_, doc_):
    """Per-(chunk, range) padded index-count quotas, max over cores."""
    n_chunks = 1 + B_SH // DOC_ROWS_PER_CHUNK
    counts = np.zeros((N_CORES, n_chunks, NRANGES), np.int64)
    for c in range(N_CORES):
        for k, toks in enumerate(_core_chunk_tokens(query_, doc_, c)):
            for r in range(NRANGES):
                counts[c, k, r] = int(((toks >> 15) == r).sum())
    quotas = tuple(
        tuple(int(_r128(counts[:, k, r].max())) for r in range(NRANGES))
        for k in range(n_chunks)
    )
    return quotas


def _wrap16(arr):
    """dma_gather index layout: [16, n/16] s-major, replicated to 128 rows."""
    n = len(arr)
    assert n % 16 == 0
    w = arr.reshape(n // 16, 16).T.astype(np.int16)
    return np.tile(w, (8, 1))


def _chunk_index_arrays(toks, quota_row):
    """Range-sort one chunk's tokens; return (gidx cols, perm wrapped)."""
    order = np.argsort(toks >> 15, kind="stable")
    sorted_toks = toks[order]
    ranges = sorted_toks >> 15
    pos_of_sorted = np.zeros(len(toks), np.int64)
    parts = []
    s0 = 0
    for r in range(NRANGES):
        q = quota_row[r]
        if q == 0:
            continue
        sel = np.nonzero(ranges == r)[0]
        cnt = len(sel)
        assert cnt <= q
        loc = (sorted_toks[sel] - RANGE * r).astype(np.int16)
        parts.append(np.concatenate([loc, np.zeros(q - cnt, np.int16)]))
        pos_of_sorted[sel] = s0 + np.arange(cnt)
        s0 += q
    perm = np.zeros(len(toks), np.int64)
    perm[order] = pos_of_sorted
    gidx = np.concatenate([_wrap16(p) for p in parts], axis=1)
    pidx = _wrap16(perm)
    return gidx, pidx


def build_core_kernel(nc, tc, quotas, b_sh=B_SH):
    n_chunks = len(quotas)
    sg_cols = sum(sum(q for q in row) for row in quotas) // 16

    # ---- DRAM I/O
    emb_d = nc.dram_tensor("emb", [V, H], F16, kind="ExternalInput").ap()
    gidx_d = nc.dram_tensor("gidx", [128, sg_cols], I16, kind="ExternalInput").ap()
    pidx_d = nc.dram_tensor(
        "pidx", [128, n_chunks * CHUNK_TOK // 16], I16, kind="ExternalInput").ap()
    qw_d = nc.dram_tensor("qw", [24, 128, 128], F16, kind="ExternalInput").ap()
    dw_d = nc.dram_tensor("dw", [24, 128, 128], F16, kind="ExternalInput").ap()
    qsw_d = nc.dram_tensor("qsw", [4, 128, 128], F32, kind="ExternalInput").ap()
    dsw_d = nc.dram_tensor("dsw", [4, 128, 128], F32, kind="ExternalInput").ap()
    qcb_d = nc.dram_tensor("qcb", [128, 4], F32, kind="ExternalInput").ap()
    dcb_d = nc.dram_tensor("dcb", [128, 4], F32, kind="ExternalInput").ap()
    qsb_d = nc.dram_tensor("qsb", [128, 1], F32, kind="ExternalInput").ap()
    dsb_d = nc.dram_tensor("dsb", [128, 1], F32, kind="ExternalInput").ap()
    gam_d = nc.dram_tensor("gam", [128, 6], F32, kind="ExternalInput").ap()
    gb_d = nc.dram_tensor("gb", [1, 1], F32, kind="ExternalInput").ap()
    out_d = nc.dram_tensor("out", [1, b_sh], F32, kind="ExternalOutput").ap()

    with ExitStack() as ctx:
        const_p = ctx.enter_context(tc.tile_pool(name="const", bufs=1))
        state_p = ctx.enter_context(tc.tile_pool(name="state", bufs=1))
        xraw_p = ctx.enter_context(tc.tile_pool(name="xraw", bufs=2))
        xt_p = ctx.enter_context(tc.tile_pool(name="xt", bufs=2))
        cval_p = ctx.enter_context(tc.tile_pool(name="cval", bufs=4))
        work_p = ctx.enter_context(tc.tile_pool(name="work", bufs=2))
        psum_p = ctx.enter_context(tc.tile_pool(name="psum", bufs=4, space="PSUM"))
        psum2_p = ctx.enter_context(tc.tile_pool(name="psum2", bufs=2, space="PSUM"))
        psum3_p = ctx.enter_context(tc.tile_pool(name="psum3", bufs=1, space="PSUM"))

        # ---- index tensors / weights to SBUF
        gidx_sb = const_p.tile([128, sg_cols], I16)
        nc.sync.dma_start(gidx_sb[:, :], gidx_d[:, :])
        pidx_sb = const_p.tile([128, n_chunks * CHUNK_TOK // 16], I16)
        nc.sync.dma_start(pidx_sb[:, :], pidx_d[:, :])

        w_sb = {}
        for br, wd in (("q", qw_d), ("d", dw_d)):
            t = const_p.tile([128, 24, 128], F16, tag=f"w_{br}", name=f"w_{br}")
            nc.sync.dma_start(t[:, :, :], wd[:, :, :].rearrange("t p c -> p t c"))
            w_sb[br] = t

        semw_sb = {}
        for br, wd in (("q", qsw_d), ("d", dsw_d)):
            t = const_p.tile([128, 4, 128], F32, tag=f"semw_{br}", name=f"semw_{br}")
            nc.sync.dma_start(t[:, :, :], wd[:, :, :].rearrange("t p c -> p t c"))
            semw_sb[br] = t

        cb_sb = {}
        for br, bd in (("q", qcb_d), ("d", dcb_d)):
            t = const_p.tile([128, 4], F32, tag=f"cb_{br}", name=f"cb_{br}")
            nc.sync.dma_start(t[:, :], bd[:, :])
            cb_sb[br] = t

        sb_sb = {}
        for br, bd in (("q", qsb_d), ("d", dsb_d)):
            t = const_p.tile([128, 1], F32, tag=f"sb_{br}", name=f"sb_{br}")
            nc.sync.dma_start(t[:, :], bd[:, :])
            sb_sb[br] = t

        gam_sb = const_p.tile([128, 6], F32)
        nc.sync.dma_start(gam_sb[:, :], gam_d[:, :])
        gb_sb = const_p.tile([1, 1], F32)
        nc.sync.dma_start(gb_sb[:, :], gb_d[:, :])

        ones_sb = const_p.tile([128, 1], F32)
        nc.vector.memset(ones_sb[:, :], 1.0)

        # ---- per-branch top-8 state: values + indices per (m, row)
        topv = {}
        topi = {}
        for br in ("q", "d"):
            topv[br] = state_p.tile([128, 4, b_sh, 8], F32,
                                    tag=f"topv_{br}", name=f"topv_{br}")
            topi[br] = state_p.tile([128, 4, b_sh, 8], U32,
                                    tag=f"topi_{br}", name=f"topi_{br}")

        # chunk list: (branch, n_rows, row0, l_row)
        chunk_info = [("q", b_sh, 0, QL)]
        for r0 in range(0, b_sh, DOC_ROWS_PER_CHUNK):
            chunk_info.append(("d", DOC_ROWS_PER_CHUNK, r0, DL))

        gcol = 0
        for ci, (br, nrows, row0, l_row) in enumerate(chunk_info):
            l_out = l_row - KS + 1
            lk = sum(quotas[ci])

            # 1a. per-range gather HBM -> SBUF (range-sorted token rows)
            xraw = xraw_p.tile([128, lk // 128, H], F16, tag="xraw")
            s0 = 0
            for r in range(NRANGES):
                q = quotas[ci][r]
                if q == 0:
                    continue
                nc.gpsimd.dma_gather(
                    out_ap=xraw[:, s0 // 128:(s0 + q) // 128, :],
                    in_ap=emb_d[RANGE * r:, :],
                    idxs_ap=gidx_sb[:, gcol:gcol + q // 16],
                    num_idxs=q,
                    num_idxs_reg=q,
                    elem_size=H,
                    single_packet=False,
                )
                gcol += q // 16
                s0 += q

            # 1b. permutation + transpose gather SBUF -> SBUF
            xt = xt_p.tile([128, 2, CHUNK_TOK], F16, tag="xt")
            nc.gpsimd.dma_gather(
                out_ap=xt[:, :, :],
                in_ap=xraw[:, :, :].rearrange("p a b -> p (a b)"),
                idxs_ap=pidx_sb[:, ci * CHUNK_TOK // 16:(ci + 1) * CHUNK_TOK // 16],
                num_idxs=CHUNK_TOK,
                num_idxs_reg=CHUNK_TOK,
                elem_size=H,
                transpose=True,
                sbuf_tokens_per_rank=128,
                sbuf_free_dim_per_rank=H * 2,
                single_packet=False,
            )

            # rows-per-matmul so N = rb*l_out <= 512
            rb = max(1, 512 // l_out)
            rb = min(rb, nrows)
            n_bc = -(-nrows // rb)

            # 2+3. conv matmuls + psum copy + topk
            for m in range(4):
                for bc in range(n_bc):
                    r = bc * rb
                    nrb = min(rb, nrows - r)
                    nn = nrb * l_out
                    ps = psum_p.tile([128, 512], F32, tag="conv_ps")
                    for k2 in range(2):
                        xtv = xt[:, k2, :].rearrange("h (r l) -> h r l", l=l_row)
                        for j in range(KS):
                            nc.tensor.matmul(
                                ps[:, :nn],
                                w_sb[br][:, m * 6 + k2 * 3 + j, :],
                                xtv[:, r:r + nrb, j:j + l_out],
                                start=(k2 == 0 and j == 0),
                                stop=(k2 == 1 and j == KS - 1),
                            )
                    cval = cval_p.tile([128, 512], F32, tag="cval")
                    nc.scalar.copy(cval[:, :nn], ps[:, :nn])
                    for rr in range(nrb):
                        row = row0 + r + rr
                        vals = cval[:, rr * l_out:(rr + 1) * l_out]
                        nc.vector.max(out=topv[br][:, m, row, :], in_=vals)
                        nc.vector.max_index(
                            out=topi[br][:, m, row, :],
                            in_max=topv[br][:, m, row, :],
                            in_values=vals,
                        )

        # ---- 4. restore positional order of top-3, bias+tanh, sem matmul
        s_sb = {}
        for br in ("q", "d"):
            ps_s = psum2_p.tile([128, KMAX * b_sh], F32, tag="sem_ps")
            for m in range(4):
                i0 = topi[br][:, m, :, 0]
                i1 = topi[br][:, m, :, 1]
                i2 = topi[br][:, m, :, 2]
                v0 = topv[br][:, m, :, 0]
                v1 = topv[br][:, m, :, 1]
                v2 = topv[br][:, m, :, 2]

                wk = work_p.tile([128, 8, b_sh], F32, tag="sortwk")
                c01, c02, c12, r0_, r1_ = (
                    wk[:, 0, :], wk[:, 1, :], wk[:, 2, :],
                    wk[:, 3, :], wk[:, 4, :],
                )
                wkm = work_p.tile([128, b_sh], U32, tag="sortmsk")
                msk = wkm[:, :]
                gt = mybir.AluOpType.is_gt
                nc.vector.tensor_tensor(out=c01, in0=i0, in1=i1, op=gt)
                nc.vector.tensor_tensor(out=c02, in0=i0, in1=i2, op=gt)
                nc.vector.tensor_tensor(out=c12, in0=i1, in1=i2, op=gt)
                # rank of value-r among indices (0 = leftmost position)
                nc.vector.tensor_tensor(
                    out=r0_, in0=c01, in1=c02, op=mybir.AluOpType.add
                )
                # r1 = (1 - c01) + c12
                nc.vector.tensor_scalar(
                    out=r1_, in0=c01, scalar1=-1.0, scalar2=1.0,
                    op0=mybir.AluOpType.mult, op1=mybir.AluOpType.add,
                )
                nc.vector.tensor_tensor(
                    out=r1_, in0=r1_, in1=c12, op=mybir.AluOpType.add
                )

                # k_all[:, s, :]: value whose rank == s; default v2 (rank r2)
                k_all = work_p.tile([128, KMAX, b_sh], F32, tag="k_all")
                eqop = mybir.AluOpType.is_equal
                for s in range(KMAX):
                    ks = k_all[:, s, :]
                    nc.vector.tensor_copy(ks, v2)
                    nc.vector.tensor_scalar(
                        out=msk, in0=r1_, scalar1=float(s), scalar2=None, op0=eqop
                    )
                    nc.vector.copy_predicated(ks, msk, v1)
                    nc.vector.tensor_scalar(
                        out=msk, in0=r0_, scalar1=float(s), scalar2=None, op0=eqop
                    )
                    nc.vector.copy_predicated(ks, msk, v0)

                # bias + tanh (fold of conv bias, monotonic-safe)
                k_tanh = work_p.tile([128, KMAX, b_sh], F32, tag="k_tanh")
                nc.scalar.activation(
                    k_tanh[:, :, :].rearrange("p k b -> p (k b)"),
                    k_all[:, :, :].rearrange("p k b -> p (k b)"),
                    mybir.ActivationFunctionType.Tanh,
                    bias=cb_sb[br][:, m:m + 1],
                )

                # sem matmul (fp32): accumulate over co tiles
                nc.tensor.matmul(
                    ps_s[:, :],
                    semw_sb[br][:, m, :],
                    k_tanh[:, :, :].rearrange("p k b -> p (k b)"),
                    start=(m == 0),
                    stop=(m == 3),
                )

            s_t = state_p.tile([128, KMAX, b_sh], F32, tag=f"s_{br}",
                               name=f"s_{br}")
            nc.scalar.activation(
                s_t[:, :, :].rearrange("p k b -> p (k b)"),
                ps_s[:, :],
                mybir.ActivationFunctionType.Tanh,
                bias=sb_sb[br][:, :],
            )
            s_sb[br] = s_t

        # ---- 5. gamma projection: acc[p, b] = sum_{br,k} s*g ; out = ones.T@acc
        acc = state_p.tile([128, b_sh], F32)
        tmp = state_p.tile([128, b_sh], F32)
        first = True
        for bi, br in enumerate(("q", "d")):
            for k in range(KMAX):
                dst = acc if first else tmp
                nc.vector.tensor_tensor(
                    out=dst[:, :],
                    in0=s_sb[br][:, k, :],
                    in1=gam_sb[:, bi * 3 + k:bi * 3 + k + 1].to_broadcast(
                        [128, b_sh]
                    ),
                    op=mybir.AluOpType.mult,
                )
                if not first:
                    nc.vector.tensor_tensor(
                        out=acc[:, :], in0=acc[:, :], in1=tmp[:, :],
                        op=mybir.AluOpType.add,
                    )
                first = False

        ps_f = psum3_p.tile([128, b_sh], F32, tag="fin_ps")
        nc.tensor.matmul(
            ps_f[0:1, :], ones_sb[:, :], acc[:, :], start=True, stop=True
        )
        res = state_p.tile([1, b_sh], F32)
        nc.vector.tensor_tensor(
            out=res[:, :], in0=ps_f[0:1, :],
            in1=gb_sb[:, :].to_broadcast([1, b_sh]), op=mybir.AluOpType.add,
        )
        nc.sync.dma_start(out_d[:, :], res[:, :])


def prep_inputs_for_core(core, quotas, query_, doc_, emb, qconv_w, qconv_b,
                         dconv_w, dconv_b, qsem_w, qsem_b, dsem_w, dsem_b,
                         gamma_w, gamma_b, b_sh=B_SH):
    gidx_parts = []
    pidx_parts = []
    for k, toks in enumerate(_core_chunk_tokens(query_, doc_, core, b_sh)):
        g, p = _chunk_index_arrays(toks, quotas[k])
        gidx_parts.append(g)
        pidx_parts.append(p)

    def prep_conv(w):
        arr = np.asarray(w).reshape(4, 128, 2, 128, KS).transpose(0, 2, 4, 3, 1)
        return np.ascontiguousarray(arr).reshape(24, 128, 128).astype(np.float16)

    def prep_sem(w):
        return np.ascontiguousarray(
            np.asarray(w).T.reshape(4, 128, 128)
        ).astype(np.float32)

    return {
        "emb": np.asarray(emb, dtype=np.float32).astype(np.float16),
        "gidx": np.concatenate(gidx_parts, axis=1),
        "pidx": np.concatenate(pidx_parts, axis=1),
        "qw": prep_conv(qconv_w),
        "dw": prep_conv(dconv_w),
        "qsw": prep_sem(qsem_w),
        "dsw": prep_sem(dsem_w),
        "qcb": np.ascontiguousarray(
            np.asarray(qconv_b).reshape(4, 128).T
        ).astype(np.float32),
        "dcb": np.ascontiguousarray(
            np.asarray(dconv_b).reshape(4, 128).T
        ).astype(np.float32),
        "qsb": np.asarray(qsem_b, dtype=np.float32).reshape(128, 1),
        "dsb": np.asarray(dsem_b, dtype=np.float32).reshape(128, 1),
        "gam": np.ascontiguousarray(
            np.asarray(gamma_w, dtype=np.float32).reshape(6, 128).T
        ),
        "gb": np.asarray(gamma_b, dtype=np.float32).reshape(1, 1),
    }


_MODULE_CACHE = {}


def get_module(quotas, b_sh=B_SH):
    key = (quotas, b_sh)
    if key not in _MODULE_CACHE:
        nc = bacc.Bacc(
            "TRN2",
            target_bir_lowering=False,
            debug=False,
            enable_asserts=False,
            num_devices=N_CORES,
        )
        with tile.TileContext(nc) as tc:
            build_core_kernel(nc, tc, quotas, b_sh=b_sh)
        nc.compile()
        _MODULE_CACHE[key] = nc
    return _MODULE_CACHE[key]


def run_cores(inputs, trace=False, **kw):
    quotas = plan_quotas(inputs["query_"], inputs["doc_"])
    nc = get_module(quotas)
    in_maps = [prep_inputs_for_core(c, quotas, **inputs) for c in range(N_CORES)]
    res = run_bass_kernel_spmd(
        nc, in_maps, core_ids=list(range(N_CORES)), trace=trace, **kw
    )
    out = np.concatenate(
        [res.results[c]["out"].reshape(B_SH, 1) for c in range(N_CORES)], axis=0
    )
    return out.astype(np.float32), res


def kernel(**inputs) -> np.ndarray:
    out, _ = run_cores(inputs, trace=False)
    return out


# revision 10
# speedup vs baseline: 1.2315x; 1.2315x over previous
"""DSSM (conv + kmax-pooling + sem) Trainium2 kernel.

Data-parallel over batch across 8 NeuronCores; weights and the embedding
table (host-cast to fp16) are replicated per core.  Each core:
  1. two-stage embedding gather:
     a) per-vocab-range `dma_gather` HBM->SBUF (int16 indices; tokens
        range-sorted host-side, ranges padded to 128 with token 0),
     b) one SBUF-source transposed `dma_gather` whose permutation indices
        simultaneously undo the range-sort and transpose to [H, tok] with
        tokens contiguous per batch row,
  2. conv1d == 6 accumulated matmuls (2 H-tiles x 3 shifts) in fp16,
  3. k-max pooling via DVE max/max_index (top-8) + branchless rank/select
     to restore positional order (bias/tanh folded to after the pooling --
     both are monotonic along L),
  4. sem layer as fp32 matmul, tanh on ACT, gamma projection via a
     ones-vector matmul partition-reduce.
"""

from contextlib import ExitStack

import numpy as np

import concourse.bass as bass
import concourse.tile as tile
from concourse import bacc, mybir
from concourse.bass_utils import run_bass_kernel_spmd

# problem dims
B, QL, DL = 512, 64, 256
V, H = 100000, 256
CO, KS = 512, 3
KMAX, LAT = 3, 128
N_CORES = 8
B_SH = B // N_CORES          # 64 rows per core
RANGE = 1 << 15              # int16-addressable vocab range
NRANGES = -(-V // RANGE)
DOC_ROWS_PER_CHUNK = 16
CHUNK_TOK = 4096             # query: 64x64; doc chunk: 16x256

F32 = mybir.dt.float32
F16 = mybir.dt.float16
I16 = mybir.dt.int16
U32 = mybir.dt.uint32


def _r128(n):
    return -(-n // 128) * 128


def _core_chunk_tokens(query_, doc_, core, b_sh=B_SH):
    """Token id arrays (CHUNK_TOK each) for each chunk of one core."""
    qs = np.asarray(query_[core * b_sh:(core + 1) * b_sh]).astype(np.int64)
    ds = np.asarray(doc_[core * b_sh:(core + 1) * b_sh]).astype(np.int64)
    chunks = [qs.ravel()]
    for r0 in range(0, b_sh, DOC_ROWS_PER_CHUNK):
        chunks.append(ds[r0:r0 + DOC_ROWS_PER_CHUNK].ravel())
    return chunks


def plan_quotas(query_, doc_):
    """Per-(chunk, range) padded index-count quotas, max over cores."""
    n_chunks = 1 + B_SH // DOC_ROWS_PER_CHUNK
    counts = np.zeros((N_CORES, n_chunks, NRANGES), np.int64)
    for c in range(N_CORES):
        for k, toks in enumerate(_core_chunk_tokens(query_, doc_, c)):
            for r in range(NRANGES):
                counts[c, k, r] = int(((toks >> 15) == r).sum())
    quotas = tuple(
        tuple(int(_r128(counts[:, k, r].max())) for r in range(NRANGES))
        for k in range(n_chunks)
    )
    return quotas


def _wrap16(arr):
    """dma_gather index layout: [16, n/16] s-major, replicated to 128 rows."""
    n = len(arr)
    assert n % 16 == 0
    w = arr.reshape(n // 16, 16).T.astype(np.int16)
    return np.tile(w, (8, 1))


def _chunk_index_arrays(toks, quota_row):
    """Range-sort one chunk's tokens; return (gidx cols, perm wrapped)."""
    order = np.argsort(toks >> 15, kind="stable")
    sorted_toks = toks[order]
    ranges = sorted_toks >> 15
    pos_of_sorted = np.zeros(len(toks), np.int64)
    parts = []
    s0 = 0
    for r in range(NRANGES):
        q = quota_row[r]
        if q == 0:
            continue
        sel = np.nonzero(ranges == r)[0]
        cnt = len(sel)
        assert cnt <= q
        loc = (sorted_toks[sel] - RANGE * r).astype(np.int16)
        parts.append(np.concatenate([loc, np.zeros(q - cnt, np.int16)]))
        pos_of_sorted[sel] = s0 + np.arange(cnt)
        s0 += q
    perm = np.zeros(len(toks), np.int64)
    perm[order] = pos_of_sorted
    gidx = np.concatenate([_wrap16(p) for p in parts], axis=1)
    pidx = _wrap16(perm)
    return gidx, pidx


def build_core_kernel(nc, tc, quotas, b_sh=B_SH):
    n_chunks = len(quotas)
    sg_cols = sum(sum(q for q in row) for row in quotas) // 16

    # ---- DRAM I/O
    emb_d = nc.dram_tensor("emb", [V, H], F16, kind="ExternalInput").ap()
    gidx_d = nc.dram_tensor("gidx", [128, sg_cols], I16, kind="ExternalInput").ap()
    pidx_d = nc.dram_tensor(
        "pidx", [128, n_chunks * CHUNK_TOK // 16], I16, kind="ExternalInput").ap()
    qw_d = nc.dram_tensor("qw", [24, 128, 128], F16, kind="ExternalInput").ap()
    dw_d = nc.dram_tensor("dw", [24, 128, 128], F16, kind="ExternalInput").ap()
    qsw_d = nc.dram_tensor("qsw", [4, 128, 128], F32, kind="ExternalInput").ap()
    dsw_d = nc.dram_tensor("dsw", [4, 128, 128], F32, kind="ExternalInput").ap()
    qcb_d = nc.dram_tensor("qcb", [128, 4], F32, kind="ExternalInput").ap()
    dcb_d = nc.dram_tensor("dcb", [128, 4], F32, kind="ExternalInput").ap()
    qsb_d = nc.dram_tensor("qsb", [128, 1], F32, kind="ExternalInput").ap()
    dsb_d = nc.dram_tensor("dsb", [128, 1], F32, kind="ExternalInput").ap()
    gam_d = nc.dram_tensor("gam", [128, 6], F32, kind="ExternalInput").ap()
    gb_d = nc.dram_tensor("gb", [1, 1], F32, kind="ExternalInput").ap()
    out_d = nc.dram_tensor("out", [1, b_sh], F32, kind="ExternalOutput").ap()

    with ExitStack() as ctx:
        const_p = ctx.enter_context(tc.tile_pool(name="const", bufs=1))
        state_p = ctx.enter_context(tc.tile_pool(name="state", bufs=1))
        xraw_p = ctx.enter_context(tc.tile_pool(name="xraw", bufs=2))
        xt_p = ctx.enter_context(tc.tile_pool(name="xt", bufs=2))
        cval_p = ctx.enter_context(tc.tile_pool(name="cval", bufs=4))
        work_p = ctx.enter_context(tc.tile_pool(name="work", bufs=2))
        psum_p = ctx.enter_context(tc.tile_pool(name="psum", bufs=4, space="PSUM"))
        psum2_p = ctx.enter_context(tc.tile_pool(name="psum2", bufs=2, space="PSUM"))
        psum3_p = ctx.enter_context(tc.tile_pool(name="psum3", bufs=1, space="PSUM"))

        # ---- index tensors / weights to SBUF
        gidx_sb = const_p.tile([128, sg_cols], I16)
        nc.sync.dma_start(gidx_sb[:, :], gidx_d[:, :])
        pidx_sb = const_p.tile([128, n_chunks * CHUNK_TOK // 16], I16)
        nc.sync.dma_start(pidx_sb[:, :], pidx_d[:, :])

        w_sb = {}
        for br, wd in (("q", qw_d), ("d", dw_d)):
            t = const_p.tile([128, 24, 128], F16, tag=f"w_{br}", name=f"w_{br}")
            nc.sync.dma_start(t[:, :, :], wd[:, :, :].rearrange("t p c -> p t c"))
            w_sb[br] = t

        semw_sb = {}
        for br, wd in (("q", qsw_d), ("d", dsw_d)):
            t = const_p.tile([128, 4, 128], F32, tag=f"semw_{br}", name=f"semw_{br}")
            nc.sync.dma_start(t[:, :, :], wd[:, :, :].rearrange("t p c -> p t c"))
            semw_sb[br] = t

        cb_sb = {}
        for br, bd in (("q", qcb_d), ("d", dcb_d)):
            t = const_p.tile([128, 4], F32, tag=f"cb_{br}", name=f"cb_{br}")
            nc.sync.dma_start(t[:, :], bd[:, :])
            cb_sb[br] = t

        sb_sb = {}
        for br, bd in (("q", qsb_d), ("d", dsb_d)):
            t = const_p.tile([128, 1], F32, tag=f"sb_{br}", name=f"sb_{br}")
            nc.sync.dma_start(t[:, :], bd[:, :])
            sb_sb[br] = t

        gam_sb = const_p.tile([128, 6], F32)
        nc.sync.dma_start(gam_sb[:, :], gam_d[:, :])
        gb_sb = const_p.tile([1, 1], F32)
        nc.sync.dma_start(gb_sb[:, :], gb_d[:, :])

        ones_sb = const_p.tile([128, 1], F32)
        nc.vector.memset(ones_sb[:, :], 1.0)

        # ---- per-branch top-8 state: values + indices per (m, row)
        topv = {}
        topi = {}
        for br in ("q", "d"):
            topv[br] = state_p.tile([128, 4, b_sh, 8], F32,
                                    tag=f"topv_{br}", name=f"topv_{br}")
            topi[br] = state_p.tile([128, 4, b_sh, 8], U32,
                                    tag=f"topi_{br}", name=f"topi_{br}")

        # chunk list: (branch, n_rows, row0, l_row)
        chunk_info = [("q", b_sh, 0, QL)]
        for r0 in range(0, b_sh, DOC_ROWS_PER_CHUNK):
            chunk_info.append(("d", DOC_ROWS_PER_CHUNK, r0, DL))

        gcol = 0
        qrr = 0
        for ci, (br, nrows, row0, l_row) in enumerate(chunk_info):
            l_out = l_row - KS + 1
            lk = sum(quotas[ci])

            # 1a. per-range gather HBM -> SBUF (range-sorted token rows)
            xraw = xraw_p.tile([128, lk // 128, H], F16, tag="xraw")
            s0 = 0
            for r in range(NRANGES):
                q = quotas[ci][r]
                if q == 0:
                    continue
                nc.gpsimd.dma_gather(
                    out_ap=xraw[:, s0 // 128:(s0 + q) // 128, :],
                    in_ap=emb_d[RANGE * r:, :],
                    idxs_ap=gidx_sb[:, gcol:gcol + q // 16],
                    num_idxs=q,
                    num_idxs_reg=q,
                    elem_size=H,
                    single_packet=False,
                    queue_num=qrr % 4,
                )
                qrr += 1
                gcol += q // 16
                s0 += q

            # 1b. permutation + transpose gather SBUF -> SBUF
            xt = xt_p.tile([128, 2, CHUNK_TOK], F16, tag="xt")
            nc.gpsimd.dma_gather(
                out_ap=xt[:, :, :],
                in_ap=xraw[:, :, :].rearrange("p a b -> p (a b)"),
                idxs_ap=pidx_sb[:, ci * CHUNK_TOK // 16:(ci + 1) * CHUNK_TOK // 16],
                num_idxs=CHUNK_TOK,
                num_idxs_reg=CHUNK_TOK,
                elem_size=H,
                transpose=True,
                sbuf_tokens_per_rank=128,
                sbuf_free_dim_per_rank=H * 2,
                single_packet=False,
                queue_num=qrr % 4,
            )
            qrr += 1

            # rows-per-matmul so N = rb*l_out <= 512
            rb = max(1, 512 // l_out)
            rb = min(rb, nrows)
            n_bc = -(-nrows // rb)

            # 2+3. conv matmuls + psum copy + topk
            for m in range(4):
                for bc in range(n_bc):
                    r = bc * rb
                    nrb = min(rb, nrows - r)
                    nn = nrb * l_out
                    ps = psum_p.tile([128, 512], F32, tag="conv_ps")
                    for k2 in range(2):
                        xtv = xt[:, k2, :].rearrange("h (r l) -> h r l", l=l_row)
                        for j in range(KS):
                            nc.tensor.matmul(
                                ps[:, :nn],
                                w_sb[br][:, m * 6 + k2 * 3 + j, :],
                                xtv[:, r:r + nrb, j:j + l_out],
                                start=(k2 == 0 and j == 0),
                                stop=(k2 == 1 and j == KS - 1),
                            )
                    cval = cval_p.tile([128, 512], F32, tag="cval")
                    nc.scalar.copy(cval[:, :nn], ps[:, :nn])
                    for rr in range(nrb):
                        row = row0 + r + rr
                        vals = cval[:, rr * l_out:(rr + 1) * l_out]
                        nc.vector.max(out=topv[br][:, m, row, :], in_=vals)
                        nc.vector.max_index(
                            out=topi[br][:, m, row, :],
                            in_max=topv[br][:, m, row, :],
                            in_values=vals,
                        )

        # ---- 4. restore positional order of top-3, bias+tanh, sem matmul
        s_sb = {}
        for br in ("q", "d"):
            ps_s = psum2_p.tile([128, KMAX * b_sh], F32, tag="sem_ps")
            for m in range(4):
                i0 = topi[br][:, m, :, 0]
                i1 = topi[br][:, m, :, 1]
                i2 = topi[br][:, m, :, 2]
                v0 = topv[br][:, m, :, 0]
                v1 = topv[br][:, m, :, 1]
                v2 = topv[br][:, m, :, 2]

                wk = work_p.tile([128, 8, b_sh], F32, tag="sortwk")
                c01, c02, c12, r0_, r1_ = (
                    wk[:, 0, :], wk[:, 1, :], wk[:, 2, :],
                    wk[:, 3, :], wk[:, 4, :],
                )
                wkm = work_p.tile([128, b_sh], U32, tag="sortmsk")
                msk = wkm[:, :]
                gt = mybir.AluOpType.is_gt
                nc.vector.tensor_tensor(out=c01, in0=i0, in1=i1, op=gt)
                nc.vector.tensor_tensor(out=c02, in0=i0, in1=i2, op=gt)
                nc.vector.tensor_tensor(out=c12, in0=i1, in1=i2, op=gt)
                # rank of value-r among indices (0 = leftmost position)
                nc.vector.tensor_tensor(
                    out=r0_, in0=c01, in1=c02, op=mybir.AluOpType.add
                )
                # r1 = (1 - c01) + c12
                nc.vector.tensor_scalar(
                    out=r1_, in0=c01, scalar1=-1.0, scalar2=1.0,
                    op0=mybir.AluOpType.mult, op1=mybir.AluOpType.add,
                )
                nc.vector.tensor_tensor(
                    out=r1_, in0=r1_, in1=c12, op=mybir.AluOpType.add
                )

                # k_all[:, s, :]: value whose rank == s; default v2 (rank r2)
                k_all = work_p.tile([128, KMAX, b_sh], F32, tag="k_all")
                eqop = mybir.AluOpType.is_equal
                for s in range(KMAX):
                    ks = k_all[:, s, :]
                    nc.vector.tensor_copy(ks, v2)
                    nc.vector.tensor_scalar(
                        out=msk, in0=r1_, scalar1=float(s), scalar2=None, op0=eqop
                    )
                    nc.vector.copy_predicated(ks, msk, v1)
                    nc.vector.tensor_scalar(
                        out=msk, in0=r0_, scalar1=float(s), scalar2=None, op0=eqop
                    )
                    nc.vector.copy_predicated(ks, msk, v0)

                # bias + tanh (fold of conv bias, monotonic-safe)
                k_tanh = work_p.tile([128, KMAX, b_sh], F32, tag="k_tanh")
                nc.scalar.activation(
                    k_tanh[:, :, :].rearrange("p k b -> p (k b)"),
                    k_all[:, :, :].rearrange("p k b -> p (k b)"),
                    mybir.ActivationFunctionType.Tanh,
                    bias=cb_sb[br][:, m:m + 1],
                )

                # sem matmul (fp32): accumulate over co tiles
                nc.tensor.matmul(
                    ps_s[:, :],
                    semw_sb[br][:, m, :],
                    k_tanh[:, :, :].rearrange("p k b -> p (k b)"),
                    start=(m == 0),
                    stop=(m == 3),
                )

            s_t = state_p.tile([128, KMAX, b_sh], F32, tag=f"s_{br}",
                               name=f"s_{br}")
            nc.scalar.activation(
                s_t[:, :, :].rearrange("p k b -> p (k b)"),
                ps_s[:, :],
                mybir.ActivationFunctionType.Tanh,
                bias=sb_sb[br][:, :],
            )
            s_sb[br] = s_t

        # ---- 5. gamma projection: acc[p, b] = sum_{br,k} s*g ; out = ones.T@acc
        acc = state_p.tile([128, b_sh], F32)
        tmp = state_p.tile([128, b_sh], F32)
        first = True
        for bi, br in enumerate(("q", "d")):
            for k in range(KMAX):
                dst = acc if first else tmp
                nc.vector.tensor_tensor(
                    out=dst[:, :],
                    in0=s_sb[br][:, k, :],
                    in1=gam_sb[:, bi * 3 + k:bi * 3 + k + 1].to_broadcast(
                        [128, b_sh]
                    ),
                    op=mybir.AluOpType.mult,
                )
                if not first:
                    nc.vector.tensor_tensor(
                        out=acc[:, :], in0=acc[:, :], in1=tmp[:, :],
                        op=mybir.AluOpType.add,
                    )
                first = False

        ps_f = psum3_p.tile([128, b_sh], F32, tag="fin_ps")
        nc.tensor.matmul(
            ps_f[0:1, :], ones_sb[:, :], acc[:, :], start=True, stop=True
        )
        res = state_p.tile([1, b_sh], F32)
        nc.vector.tensor_tensor(
            out=res[:, :], in0=ps_f[0:1, :],
            in1=gb_sb[:, :].to_broadcast([1, b_sh]), op=mybir.AluOpType.add,
        )
        nc.sync.dma_start(out_d[:, :], res[:, :])


def prep_inputs_for_core(core, quotas, query_, doc_, emb, qconv_w, qconv_b,
                         dconv_w, dconv_b, qsem_w, qsem_b, dsem_w, dsem_b,
                         gamma_w, gamma_b, b_sh=B_SH):
    gidx_parts = []
    pidx_parts = []
    for k, toks in enumerate(_core_chunk_tokens(query_, doc_, core, b_sh)):
        g, p = _chunk_index_arrays(toks, quotas[k])
        gidx_parts.append(g)
        pidx_parts.append(p)

    def prep_conv(w):
        arr = np.asarray(w).reshape(4, 128, 2, 128, KS).transpose(0, 2, 4, 3, 1)
        return np.ascontiguousarray(arr).reshape(24, 128, 128).astype(np.float16)

    def prep_sem(w):
        return np.ascontiguousarray(
            np.asarray(w).T.reshape(4, 128, 128)
        ).astype(np.float32)

    return {
        "emb": np.asarray(emb, dtype=np.float32).astype(np.float16),
        "gidx": np.concatenate(gidx_parts, axis=1),
        "pidx": np.concatenate(pidx_parts, axis=1),
        "qw": prep_conv(qconv_w),
        "dw": prep_conv(dconv_w),
        "qsw": prep_sem(qsem_w),
        "dsw": prep_sem(dsem_w),
        "qcb": np.ascontiguousarray(
            np.asarray(qconv_b).reshape(4, 128).T
        ).astype(np.float32),
        "dcb": np.ascontiguousarray(
            np.asarray(dconv_b).reshape(4, 128).T
        ).astype(np.float32),
        "qsb": np.asarray(qsem_b, dtype=np.float32).reshape(128, 1),
        "dsb": np.asarray(dsem_b, dtype=np.float32).reshape(128, 1),
        "gam": np.ascontiguousarray(
            np.asarray(gamma_w, dtype=np.float32).reshape(6, 128).T
        ),
        "gb": np.asarray(gamma_b, dtype=np.float32).reshape(1, 1),
    }


_MODULE_CACHE = {}


def get_module(quotas, b_sh=B_SH):
    key = (quotas, b_sh)
    if key not in _MODULE_CACHE:
        nc = bacc.Bacc(
            "TRN2",
            target_bir_lowering=False,
            debug=False,
            enable_asserts=False,
            num_devices=N_CORES,
            num_swdge_queues=4,
        )
        with tile.TileContext(nc) as tc:
            build_core_kernel(nc, tc, quotas, b_sh=b_sh)
        nc.compile()
        _MODULE_CACHE[key] = nc
    return _MODULE_CACHE[key]


def run_cores(inputs, trace=False, **kw):
    quotas = plan_quotas(inputs["query_"], inputs["doc_"])
    nc = get_module(quotas)
    in_maps = [prep_inputs_for_core(c, quotas, **inputs) for c in range(N_CORES)]
    res = run_bass_kernel_spmd(
        nc, in_maps, core_ids=list(range(N_CORES)), trace=trace, **kw
    )
    out = np.concatenate(
        [res.results[c]["out"].reshape(B_SH, 1) for c in range(N_CORES)], axis=0
    )
    return out.astype(np.float32), res


def kernel(**inputs) -> np.ndarray:
    out, _ = run_cores(inputs, trace=False)
    return out


# revision 11
# speedup vs baseline: 1.3521x; 1.0980x over previous
"""DSSM (conv + kmax-pooling + sem) Trainium2 kernel.

Data-parallel over batch across 8 NeuronCores; weights and the embedding
table (host-cast to fp16) are replicated per core.  Each core:
  1. two-stage embedding gather:
     a) per-vocab-range `dma_gather` HBM->SBUF (int16 indices; tokens
        range-sorted host-side, ranges padded to 128 with token 0),
     b) one SBUF-source transposed `dma_gather` whose permutation indices
        simultaneously undo the range-sort and transpose to [H, tok] with
        tokens contiguous per batch row,
  2. conv1d == 6 accumulated matmuls (2 H-tiles x 3 shifts) in fp16,
  3. k-max pooling via DVE max/max_index (top-8) + branchless rank/select
     to restore positional order (bias/tanh folded to after the pooling --
     both are monotonic along L),
  4. sem layer as fp32 matmul, tanh on ACT, gamma projection via a
     ones-vector matmul partition-reduce.
"""

from contextlib import ExitStack

import numpy as np

import concourse.bass as bass
import concourse.tile as tile
from concourse import bacc, mybir
from concourse.bass_utils import run_bass_kernel_spmd

# problem dims
B, QL, DL = 512, 64, 256
V, H = 100000, 256
CO, KS = 512, 3
KMAX, LAT = 3, 128
N_CORES = 8
B_SH = B // N_CORES          # 64 rows per core
RANGE = 1 << 15              # int16-addressable vocab range
NRANGES = -(-V // RANGE)
DOC_ROWS_PER_CHUNK = 8
Q_ROWS_PER_CHUNK = 16
CHUNK_TOK = 4096             # upper bound (xt tile size)

F32 = mybir.dt.float32
F16 = mybir.dt.float16
I16 = mybir.dt.int16
U32 = mybir.dt.uint32


def _r128(n):
    return -(-n // 128) * 128


def _core_chunk_tokens(query_, doc_, core, b_sh=B_SH):
    """Token id arrays (CHUNK_TOK each) for each chunk of one core."""
    qs = np.asarray(query_[core * b_sh:(core + 1) * b_sh]).astype(np.int64)
    ds = np.asarray(doc_[core * b_sh:(core + 1) * b_sh]).astype(np.int64)
    chunks = []
    for r0 in range(0, b_sh, Q_ROWS_PER_CHUNK):
        chunks.append(qs[r0:r0 + Q_ROWS_PER_CHUNK].ravel())
    for r0 in range(0, b_sh, DOC_ROWS_PER_CHUNK):
        chunks.append(ds[r0:r0 + DOC_ROWS_PER_CHUNK].ravel())
    return chunks


def plan_quotas(query_, doc_):
    """Per-(chunk, range) padded index-count quotas, max over cores."""
    n_chunks = B_SH // Q_ROWS_PER_CHUNK + B_SH // DOC_ROWS_PER_CHUNK
    counts = np.zeros((N_CORES, n_chunks, NRANGES), np.int64)
    for c in range(N_CORES):
        for k, toks in enumerate(_core_chunk_tokens(query_, doc_, c)):
            for r in range(NRANGES):
                counts[c, k, r] = int(((toks >> 15) == r).sum())
    quotas = tuple(
        tuple(int(_r128(counts[:, k, r].max())) for r in range(NRANGES))
        for k in range(n_chunks)
    )
    return quotas


def _wrap16(arr):
    """dma_gather index layout: [16, n/16] s-major, replicated to 128 rows."""
    n = len(arr)
    assert n % 16 == 0
    w = arr.reshape(n // 16, 16).T.astype(np.int16)
    return np.tile(w, (8, 1))


def _chunk_index_arrays(toks, quota_row):
    """Range-sort one chunk's tokens; return (gidx cols, perm wrapped)."""
    order = np.argsort(toks >> 15, kind="stable")
    sorted_toks = toks[order]
    ranges = sorted_toks >> 15
    pos_of_sorted = np.zeros(len(toks), np.int64)
    parts = []
    s0 = 0
    for r in range(NRANGES):
        q = quota_row[r]
        if q == 0:
            continue
        sel = np.nonzero(ranges == r)[0]
        cnt = len(sel)
        assert cnt <= q
        loc = (sorted_toks[sel] - RANGE * r).astype(np.int16)
        parts.append(np.concatenate([loc, np.zeros(q - cnt, np.int16)]))
        pos_of_sorted[sel] = s0 + np.arange(cnt)
        s0 += q
    perm = np.zeros(len(toks), np.int64)
    perm[order] = pos_of_sorted
    gidx = np.concatenate([_wrap16(p) for p in parts], axis=1)
    pidx = _wrap16(perm)
    return gidx, pidx


def build_core_kernel(nc, tc, quotas, b_sh=B_SH):
    n_chunks = len(quotas)
    sg_cols = sum(sum(q for q in row) for row in quotas) // 16

    # ---- DRAM I/O
    emb_d = nc.dram_tensor("emb", [V, H], F16, kind="ExternalInput").ap()
    gidx_d = nc.dram_tensor("gidx", [128, sg_cols], I16, kind="ExternalInput").ap()
    chunk_info = [("q", Q_ROWS_PER_CHUNK, r0, QL)
                  for r0 in range(0, b_sh, Q_ROWS_PER_CHUNK)]
    chunk_info += [("d", DOC_ROWS_PER_CHUNK, r0, DL)
                   for r0 in range(0, b_sh, DOC_ROWS_PER_CHUNK)]
    assert len(chunk_info) == n_chunks
    ntoks = [nr * lr for (_, nr, _, lr) in chunk_info]
    sp_cols = sum(ntoks) // 16
    pidx_d = nc.dram_tensor("pidx", [128, sp_cols], I16, kind="ExternalInput").ap()
    qw_d = nc.dram_tensor("qw", [24, 128, 128], F16, kind="ExternalInput").ap()
    dw_d = nc.dram_tensor("dw", [24, 128, 128], F16, kind="ExternalInput").ap()
    qsw_d = nc.dram_tensor("qsw", [4, 128, 128], F32, kind="ExternalInput").ap()
    dsw_d = nc.dram_tensor("dsw", [4, 128, 128], F32, kind="ExternalInput").ap()
    qcb_d = nc.dram_tensor("qcb", [128, 4], F32, kind="ExternalInput").ap()
    dcb_d = nc.dram_tensor("dcb", [128, 4], F32, kind="ExternalInput").ap()
    qsb_d = nc.dram_tensor("qsb", [128, 1], F32, kind="ExternalInput").ap()
    dsb_d = nc.dram_tensor("dsb", [128, 1], F32, kind="ExternalInput").ap()
    gam_d = nc.dram_tensor("gam", [128, 6], F32, kind="ExternalInput").ap()
    gb_d = nc.dram_tensor("gb", [1, 1], F32, kind="ExternalInput").ap()
    out_d = nc.dram_tensor("out", [1, b_sh], F32, kind="ExternalOutput").ap()

    with ExitStack() as ctx:
        const_p = ctx.enter_context(tc.tile_pool(name="const", bufs=1))
        state_p = ctx.enter_context(tc.tile_pool(name="state", bufs=1))
        xraw_p = ctx.enter_context(tc.tile_pool(name="xraw", bufs=2))
        xt_p = ctx.enter_context(tc.tile_pool(name="xt", bufs=2))
        cval_p = ctx.enter_context(tc.tile_pool(name="cval", bufs=4))
        work_p = ctx.enter_context(tc.tile_pool(name="work", bufs=2))
        psum_p = ctx.enter_context(tc.tile_pool(name="psum", bufs=4, space="PSUM"))
        psum2_p = ctx.enter_context(tc.tile_pool(name="psum2", bufs=2, space="PSUM"))
        psum3_p = ctx.enter_context(tc.tile_pool(name="psum3", bufs=1, space="PSUM"))

        # ---- index tensors / weights to SBUF
        gidx_sb = const_p.tile([128, sg_cols], I16)
        nc.sync.dma_start(gidx_sb[:, :], gidx_d[:, :])
        pidx_sb = const_p.tile([128, sp_cols], I16)
        nc.sync.dma_start(pidx_sb[:, :], pidx_d[:, :])

        w_sb = {}
        for br, wd in (("q", qw_d), ("d", dw_d)):
            t = const_p.tile([128, 24, 128], F16, tag=f"w_{br}", name=f"w_{br}")
            nc.sync.dma_start(t[:, :, :], wd[:, :, :].rearrange("t p c -> p t c"))
            w_sb[br] = t

        semw_sb = {}
        for br, wd in (("q", qsw_d), ("d", dsw_d)):
            t = const_p.tile([128, 4, 128], F32, tag=f"semw_{br}", name=f"semw_{br}")
            nc.sync.dma_start(t[:, :, :], wd[:, :, :].rearrange("t p c -> p t c"))
            semw_sb[br] = t

        cb_sb = {}
        for br, bd in (("q", qcb_d), ("d", dcb_d)):
            t = const_p.tile([128, 4], F32, tag=f"cb_{br}", name=f"cb_{br}")
            nc.sync.dma_start(t[:, :], bd[:, :])
            cb_sb[br] = t

        sb_sb = {}
        for br, bd in (("q", qsb_d), ("d", dsb_d)):
            t = const_p.tile([128, 1], F32, tag=f"sb_{br}", name=f"sb_{br}")
            nc.sync.dma_start(t[:, :], bd[:, :])
            sb_sb[br] = t

        gam_sb = const_p.tile([128, 6], F32)
        nc.sync.dma_start(gam_sb[:, :], gam_d[:, :])
        gb_sb = const_p.tile([1, 1], F32)
        nc.sync.dma_start(gb_sb[:, :], gb_d[:, :])

        ones_sb = const_p.tile([128, 1], F32)
        nc.vector.memset(ones_sb[:, :], 1.0)

        # ---- per-branch top-8 state: values + indices per (m, row)
        topv = {}
        topi = {}
        for br in ("q", "d"):
            topv[br] = state_p.tile([128, 4, b_sh, 8], F32,
                                    tag=f"topv_{br}", name=f"topv_{br}")
            topi[br] = state_p.tile([128, 4, b_sh, 8], U32,
                                    tag=f"topi_{br}", name=f"topi_{br}")

        gcol = 0
        pcol = 0
        qrr = 0
        for ci, (br, nrows, row0, l_row) in enumerate(chunk_info):
            l_out = l_row - KS + 1
            lk = sum(quotas[ci])
            ntok = ntoks[ci]

            # 1a. per-range gather HBM -> SBUF (range-sorted token rows)
            xraw = xraw_p.tile([128, lk // 128, H], F16, tag="xraw")
            s0 = 0
            for r in range(NRANGES):
                q = quotas[ci][r]
                if q == 0:
                    continue
                nc.gpsimd.dma_gather(
                    out_ap=xraw[:, s0 // 128:(s0 + q) // 128, :],
                    in_ap=emb_d[RANGE * r:, :],
                    idxs_ap=gidx_sb[:, gcol:gcol + q // 16],
                    num_idxs=q,
                    num_idxs_reg=q,
                    elem_size=H,
                    single_packet=False,
                    queue_num=qrr % 4,
                )
                qrr += 1
                gcol += q // 16
                s0 += q

            # 1b. permutation + transpose gather SBUF -> SBUF
            xt = xt_p.tile([128, 2, ntok], F16, tag="xt")
            nc.gpsimd.dma_gather(
                out_ap=xt[:, :, :],
                in_ap=xraw[:, :, :].rearrange("p a b -> p (a b)"),
                idxs_ap=pidx_sb[:, pcol:pcol + ntok // 16],
                num_idxs=ntok,
                num_idxs_reg=ntok,
                elem_size=H,
                transpose=True,
                sbuf_tokens_per_rank=128,
                sbuf_free_dim_per_rank=H * 2,
                single_packet=False,
                queue_num=qrr % 4,
            )
            qrr += 1
            pcol += ntok // 16

            # rows-per-matmul so N = rb*l_out <= 512
            rb = max(1, 512 // l_out)
            rb = min(rb, nrows)
            n_bc = -(-nrows // rb)

            # 2+3. conv matmuls + psum copy + topk
            for m in range(4):
                for bc in range(n_bc):
                    r = bc * rb
                    nrb = min(rb, nrows - r)
                    nn = nrb * l_out
                    ps = psum_p.tile([128, 512], F32, tag="conv_ps")
                    for k2 in range(2):
                        xtv = xt[:, k2, :].rearrange("h (r l) -> h r l", l=l_row)
                        for j in range(KS):
                            nc.tensor.matmul(
                                ps[:, :nn],
                                w_sb[br][:, m * 6 + k2 * 3 + j, :],
                                xtv[:, r:r + nrb, j:j + l_out],
                                start=(k2 == 0 and j == 0),
                                stop=(k2 == 1 and j == KS - 1),
                            )
                    cval = cval_p.tile([128, 512], F32, tag="cval")
                    nc.scalar.copy(cval[:, :nn], ps[:, :nn])
                    for rr in range(nrb):
                        row = row0 + r + rr
                        vals = cval[:, rr * l_out:(rr + 1) * l_out]
                        nc.vector.max(out=topv[br][:, m, row, :], in_=vals)
                        nc.vector.max_index(
                            out=topi[br][:, m, row, :],
                            in_max=topv[br][:, m, row, :],
                            in_values=vals,
                        )

        # ---- 4. restore positional order of top-3, bias+tanh, sem matmul
        s_sb = {}
        for br in ("q", "d"):
            ps_s = psum2_p.tile([128, KMAX * b_sh], F32, tag="sem_ps")
            for m in range(4):
                i0 = topi[br][:, m, :, 0]
                i1 = topi[br][:, m, :, 1]
                i2 = topi[br][:, m, :, 2]
                v0 = topv[br][:, m, :, 0]
                v1 = topv[br][:, m, :, 1]
                v2 = topv[br][:, m, :, 2]

                wk = work_p.tile([128, 8, b_sh], F32, tag="sortwk")
                c01, c02, c12, r0_, r1_ = (
                    wk[:, 0, :], wk[:, 1, :], wk[:, 2, :],
                    wk[:, 3, :], wk[:, 4, :],
                )
                wkm = work_p.tile([128, b_sh], U32, tag="sortmsk")
                msk = wkm[:, :]
                gt = mybir.AluOpType.is_gt
                nc.vector.tensor_tensor(out=c01, in0=i0, in1=i1, op=gt)
                nc.vector.tensor_tensor(out=c02, in0=i0, in1=i2, op=gt)
                nc.vector.tensor_tensor(out=c12, in0=i1, in1=i2, op=gt)
                # rank of value-r among indices (0 = leftmost position)
                nc.vector.tensor_tensor(
                    out=r0_, in0=c01, in1=c02, op=mybir.AluOpType.add
                )
                # r1 = (1 - c01) + c12
                nc.vector.tensor_scalar(
                    out=r1_, in0=c01, scalar1=-1.0, scalar2=1.0,
                    op0=mybir.AluOpType.mult, op1=mybir.AluOpType.add,
                )
                nc.vector.tensor_tensor(
                    out=r1_, in0=r1_, in1=c12, op=mybir.AluOpType.add
                )

                # k_all[:, s, :]: value whose rank == s; default v2 (rank r2)
                k_all = work_p.tile([128, KMAX, b_sh], F32, tag="k_all")
                eqop = mybir.AluOpType.is_equal
                for s in range(KMAX):
                    ks = k_all[:, s, :]
                    nc.vector.tensor_copy(ks, v2)
                    nc.vector.tensor_scalar(
                        out=msk, in0=r1_, scalar1=float(s), scalar2=None, op0=eqop
                    )
                    nc.vector.copy_predicated(ks, msk, v1)
                    nc.vector.tensor_scalar(
                        out=msk, in0=r0_, scalar1=float(s), scalar2=None, op0=eqop
                    )
                    nc.vector.copy_predicated(ks, msk, v0)

                # bias + tanh (fold of conv bias, monotonic-safe)
                k_tanh = work_p.tile([128, KMAX, b_sh], F32, tag="k_tanh")
                nc.scalar.activation(
                    k_tanh[:, :, :].rearrange("p k b -> p (k b)"),
                    k_all[:, :, :].rearrange("p k b -> p (k b)"),
                    mybir.ActivationFunctionType.Tanh,
                    bias=cb_sb[br][:, m:m + 1],
                )

                # sem matmul (fp32): accumulate over co tiles
                nc.tensor.matmul(
                    ps_s[:, :],
                    semw_sb[br][:, m, :],
                    k_tanh[:, :, :].rearrange("p k b -> p (k b)"),
                    start=(m == 0),
                    stop=(m == 3),
                )

            s_t = state_p.tile([128, KMAX, b_sh], F32, tag=f"s_{br}",
                               name=f"s_{br}")
            nc.scalar.activation(
                s_t[:, :, :].rearrange("p k b -> p (k b)"),
                ps_s[:, :],
                mybir.ActivationFunctionType.Tanh,
                bias=sb_sb[br][:, :],
            )
            s_sb[br] = s_t

        # ---- 5. gamma projection: acc[p, b] = sum_{br,k} s*g ; out = ones.T@acc
        acc = state_p.tile([128, b_sh], F32)
        tmp = state_p.tile([128, b_sh], F32)
        first = True
        for bi, br in enumerate(("q", "d")):
            for k in range(KMAX):
                dst = acc if first else tmp
                nc.vector.tensor_tensor(
                    out=dst[:, :],
                    in0=s_sb[br][:, k, :],
                    in1=gam_sb[:, bi * 3 + k:bi * 3 + k + 1].to_broadcast(
                        [128, b_sh]
                    ),
                    op=mybir.AluOpType.mult,
                )
                if not first:
                    nc.vector.tensor_tensor(
                        out=acc[:, :], in0=acc[:, :], in1=tmp[:, :],
                        op=mybir.AluOpType.add,
                    )
                first = False

        ps_f = psum3_p.tile([128, b_sh], F32, tag="fin_ps")
        nc.tensor.matmul(
            ps_f[0:1, :], ones_sb[:, :], acc[:, :], start=True, stop=True
        )
        res = state_p.tile([1, b_sh], F32)
        nc.vector.tensor_tensor(
            out=res[:, :], in0=ps_f[0:1, :],
            in1=gb_sb[:, :].to_broadcast([1, b_sh]), op=mybir.AluOpType.add,
        )
        nc.sync.dma_start(out_d[:, :], res[:, :])


def prep_inputs_for_core(core, quotas, query_, doc_, emb, qconv_w, qconv_b,
                         dconv_w, dconv_b, qsem_w, qsem_b, dsem_w, dsem_b,
                         gamma_w, gamma_b, b_sh=B_SH):
    gidx_parts = []
    pidx_parts = []
    for k, toks in enumerate(_core_chunk_tokens(query_, doc_, core, b_sh)):
        g, p = _chunk_index_arrays(toks, quotas[k])
        gidx_parts.append(g)
        pidx_parts.append(p)

    def prep_conv(w):
        arr = np.asarray(w).reshape(4, 128, 2, 128, KS).transpose(0, 2, 4, 3, 1)
        return np.ascontiguousarray(arr).reshape(24, 128, 128).astype(np.float16)

    def prep_sem(w):
        return np.ascontiguousarray(
            np.asarray(w).T.reshape(4, 128, 128)
        ).astype(np.float32)

    return {
        "emb": np.asarray(emb, dtype=np.float32).astype(np.float16),
        "gidx": np.concatenate(gidx_parts, axis=1),
        "pidx": np.concatenate(pidx_parts, axis=1),
        "qw": prep_conv(qconv_w),
        "dw": prep_conv(dconv_w),
        "qsw": prep_sem(qsem_w),
        "dsw": prep_sem(dsem_w),
        "qcb": np.ascontiguousarray(
            np.asarray(qconv_b).reshape(4, 128).T
        ).astype(np.float32),
        "dcb": np.ascontiguousarray(
            np.asarray(dconv_b).reshape(4, 128).T
        ).astype(np.float32),
        "qsb": np.asarray(qsem_b, dtype=np.float32).reshape(128, 1),
        "dsb": np.asarray(dsem_b, dtype=np.float32).reshape(128, 1),
        "gam": np.ascontiguousarray(
            np.asarray(gamma_w, dtype=np.float32).reshape(6, 128).T
        ),
        "gb": np.asarray(gamma_b, dtype=np.float32).reshape(1, 1),
    }


_MODULE_CACHE = {}


def get_module(quotas, b_sh=B_SH):
    key = (quotas, b_sh)
    if key not in _MODULE_CACHE:
        nc = bacc.Bacc(
            "TRN2",
            target_bir_lowering=False,
            debug=False,
            enable_asserts=False,
            num_devices=N_CORES,
            num_swdge_queues=4,
        )
        with tile.TileContext(nc) as tc:
            build_core_kernel(nc, tc, quotas, b_sh=b_sh)
        nc.compile()
        _MODULE_CACHE[key] = nc
    return _MODULE_CACHE[key]


def run_cores(inputs, trace=False, **kw):
    quotas = plan_quotas(inputs["query_"], inputs["doc_"])
    nc = get_module(quotas)
    in_maps = [prep_inputs_for_core(c, quotas, **inputs) for c in range(N_CORES)]
    res = run_bass_kernel_spmd(
        nc, in_maps, core_ids=list(range(N_CORES)), trace=trace, **kw
    )
    out = np.concatenate(
        [res.results[c]["out"].reshape(B_SH, 1) for c in range(N_CORES)], axis=0
    )
    return out.astype(np.float32), res


def kernel(**inputs) -> np.ndarray:
    out, _ = run_cores(inputs, trace=False)
    return out
